# revision 7
# baseline (speedup 1.0000x reference)
"""Causal single-head self-attention (B=8, S=1024, D=1024, f32) on 8 TRN2 cores.

Sharding: data-parallel over batch (1 batch element per core). The algebra is
restructured on the host to remove two of the five device GEMMs:

  scores = x wq^T wk x^T = x M x^T          with M  = wq^T wk   (host GEMM)
  y      = attn x wv^T wo^T = attn (x W2)   with W2 = wv^T wo^T (host GEMM)

Per-core dataflow (everything bf16 in SBUF, f32 PSUM accumulation):
  tT[e,i]  = M^T-slab contraction with xT        (65536 moving rows)
  v2[s,e]  = x @ W2                              (65536 rows)
  per j-tile jt (single causal pass, software-pipelined one step):
    scoresT[j,i] = x t^T  for i >= 128*jt        (36864 rows total)
    attnT = exp(scoresT/32)  (ACT, PSUM->SBUF bf16; affine_select masks the
                              diagonal 128x128 block; no other masking needed
                              because tiles are trimmed exactly to the causal
                              boundary)
    r[i]  = ones-moving matmul over attnT        ([128,1] PSUM, no transpose
                                                  or DRAM round trip)
    y[i,e] = sum_jt attnT^T @ v2, * 1/r fused into the PSUM->SBUF copy,
             streamed to DRAM per 128-row slab   (36864 rows)

bf16 matmuls run 1 cyc/row at any width (so causal trimming is exact at 128
granularity), input DMA is half of f32, and the host pre-fusion removes
~55 us of PE work vs the 5-GEMM formulation. Measured vs the fp32 reference
the scheme sits at ~4e-3 scale-relative max error (CPU bit-model), well under
the 2e-2 gate.
"""

import os
import sys

sys.path.insert(0, "/opt/trn_rl_repo")

from contextlib import ExitStack

import ml_dtypes
import numpy as np

import concourse.bass as bass
from concourse import bacc
import concourse.mybir as mybir
import concourse.tile as tile
from concourse.tile import add_dep_helper
from concourse.bass_utils import run_bass_kernel_spmd

B, S, D = 8, 1024, 1024
P = 128          # partition / stationary tile size
NB = 512         # moving-operand block (= 1 PSUM bank of f32)
NT = S // P      # 8 tiles of 128
SCALE = 1.0 / np.sqrt(float(D))

F32 = mybir.dt.float32
BF16 = mybir.dt.bfloat16

N_CORES = 8

# attnT[jt] has width 1024 - 128*jt (columns i >= 128*jt); packed offsets.
AW = [S - P * jt for jt in range(NT)]
AOFF = [sum(AW[:jt]) for jt in range(NT)]
ATOT = sum(AW)  # 4608

LAST_RESULTS = None  # BassKernelResults of the most recent run (for test.py)


def _build():
    nc = bacc.Bacc("TRN2", target_bir_lowering=False, debug=False)

    xT_d = nc.dram_tensor("xT", [D, S], BF16, kind="ExternalInput").ap()
    m_d = nc.dram_tensor("M", [D, D], BF16, kind="ExternalInput").ap()
    w2_d = nc.dram_tensor("W2", [D, D], BF16, kind="ExternalInput").ap()
    y_d = nc.dram_tensor("y", [S, D], F32, kind="ExternalOutput").ap()

    # SBUF layout of a 1024x1024 matrix: big tile [128, 8192] where column
    # range t*1024..(t+1)*1024 holds DRAM rows t*128..(t+1)*128. Each
    # dma_start costs ~600 ns of sync-queue issue time, so load multi-slab
    # chunks with a single 3D access pattern instead of per-slab transfers.
    def chunk_load(sbuf_tile, dram_ap, t0, t1):
        return nc.sync.dma_start(
            sbuf_tile[:, t0 * S : t1 * S],
            dram_ap[t0 * P : t1 * P, :].rearrange("(t p) s -> p t s", p=P),
        )

    with tile.TileContext(nc) as tc, ExitStack() as ctx:
        consts = ctx.enter_context(tc.tile_pool(name="consts", bufs=1))
        ones = consts.tile([P, 8], BF16)
        nc.gpsimd.memset(ones, 1.0)
        zbias = consts.tile([P, 1], F32)
        nc.gpsimd.memset(zbias, 0.0)
        junk = consts.tile([P, 256], BF16)
        nc.gpsimd.memset(junk, 0.5)

        psum = ctx.enter_context(tc.tile_pool(name="psum", bufs=8, space="PSUM"))

        xpool = ctx.enter_context(tc.tile_pool(name="xpool", bufs=1))
        mpool = ctx.enter_context(tc.tile_pool(name="mpool", bufs=1))
        tpool = ctx.enter_context(tc.tile_pool(name="tpool", bufs=1))
        w2pool = ctx.enter_context(tc.tile_pool(name="w2pool", bufs=1))
        vpool = ctx.enter_context(tc.tile_pool(name="vpool", bufs=1))
        apool = ctx.enter_context(tc.tile_pool(name="apool", bufs=1))
        ypool = ctx.enter_context(tc.tile_pool(name="ypool", bufs=3))
        rpool = ctx.enter_context(tc.tile_pool(name="rpool", bufs=3))

        xsb = xpool.tile([P, NT * S], BF16, name="xsb")
        msb = mpool.tile([P, NT * D], BF16, name="msb")
        tsb = tpool.tile([P, NT * S], BF16, name="tsb")
        w2sb = w2pool.tile([P, NT * D], BF16, name="w2sb")
        v2sb = vpool.tile([P, NT * D], BF16, name="v2sb")
        atile = apool.tile([P, ATOT], BF16, name="atile")

        # HAM warmup: keep the PE array busy while the first chunks are in
        # flight so the clock gate is ramped when the real waves start.
        for _ in range(8):
            pw = psum.tile([8, 256], F32, tag="mm", bufs=8, name="pw")
            nc.tensor.matmul(pw, ones, junk, start=True, stop=True)

        # Load order: x/M chunks interleaved by d-tile range; the tT phase
        # accumulates d-tile-major over 8 PSUM groups so the matmul stream
        # chases the arriving chunk pairs. Front chunks are single slabs so
        # the first matmuls start as early as possible.
        for t0, t1 in ((0, 1), (1, 2), (2, 4), (4, 6), (6, 8)):
            chunk_load(xsb, xT_d, t0, t1)
            chunk_load(msb, m_d, t0, t1)

        def mm_t(pt, et, sb, dt):
            nc.tensor.matmul(
                pt,
                msb[:, dt * D + et * P : dt * D + (et + 1) * P],
                xsb[:, dt * S + sb * NB : dt * S + (sb + 1) * NB],
                start=(dt == 0),
                stop=(dt == NT - 1),
            )

        t_copies = {}

        def t_copy(pt, et, sb):
            inst = nc.vector.tensor_copy(
                out=tsb[:, et * S + sb * NB : et * S + (sb + 1) * NB],
                in_=pt,
            )
            t_copies[(et, sb)] = inst
            return inst

        # Phase 0 of tT: 8 PSUM groups accumulated d-tile-major.
        groups = [(et, sb) for et in range(4) for sb in range(2)]
        pts = {}
        for g in groups:
            pts[g] = psum.tile([P, NB], F32, tag="mm", bufs=8, name="pt")
        for dt in range(NT):
            for (et, sb) in groups:
                mm_t(pts[(et, sb)], et, sb, dt)
        for (et, sb) in groups:
            t_copy(pts[(et, sb)], et, sb)

        # Remaining e-tiles of tT, standard order.
        for et in range(4, NT):
            for sb in range(2):
                pt = psum.tile([P, NB], F32, tag="mm", bufs=8, name="pt")
                for dt in range(NT):
                    mm_t(pt, et, sb, dt)
                t_copy(pt, et, sb)

        # W2 prefetch in two chunks spread across the tT phase.
        for i, (t0, t1) in enumerate(((0, 4), (4, 8))):
            dma = chunk_load(w2sb, w2_d, t0, t1)
            anchor = t_copies.get((1 + 2 * i, 0))
            if anchor is not None:
                add_dep_helper(dma.ins, anchor.ins, reason="w2 prefetch pacing")

        # v2[s, e] = x @ W2: stationary xT s-tile, moving W2 slab.
        for st in range(NT):
            for eb in range(2):
                pt = psum.tile([P, NB], F32, tag="mm", bufs=8, name="pt")
                for dt in range(NT):
                    nc.tensor.matmul(
                        pt,
                        xsb[:, dt * S + st * P : dt * S + (st + 1) * P],
                        w2sb[:, dt * D + eb * NB : dt * D + (eb + 1) * NB],
                        start=(dt == 0),
                        stop=(dt == NT - 1),
                    )
                nc.vector.tensor_copy(
                    out=v2sb[:, st * D + eb * NB : st * D + (eb + 1) * NB],
                    in_=pt,
                )

        def att_win(jt, it):
            return atile[:, AOFF[jt] + (it - jt) * P : AOFF[jt] + (it - jt + 1) * P]

        def scores_step(jt):
            # scoresT[j in jt, i >= 128*jt], trimmed exactly to the causal
            # boundary; exp into the packed attnT tile; mask the diagonal
            # 128x128 block.
            i0 = jt * P
            c0 = i0
            while c0 < S:
                cw = min(NB, S - c0)
                ps = psum.tile([P, cw], F32, tag="mm", bufs=8, name="ps")
                for et in range(NT):
                    nc.tensor.matmul(
                        ps,
                        xsb[:, et * S + i0 : et * S + i0 + P],
                        tsb[:, et * S + c0 : et * S + c0 + cw],
                        start=(et == 0),
                        stop=(et == NT - 1),
                    )
                nc.scalar.activation(
                    out=atile[:, AOFF[jt] + (c0 - i0) : AOFF[jt] + (c0 - i0) + cw],
                    in_=ps,
                    func=mybir.ActivationFunctionType.Exp,
                    bias=zbias,
                    scale=SCALE,
                )
                c0 += cw
            # keep where i_local - j_local >= 0 within the diagonal block
            nc.gpsimd.affine_select(
                out=atile[:, AOFF[jt] : AOFF[jt] + P],
                in_=atile[:, AOFF[jt] : AOFF[jt] + P],
                compare_op=mybir.AluOpType.is_ge,
                fill=0.0,
                base=0,
                pattern=[[1, P]],
                channel_multiplier=-1,
            )

        def ry_step(it):
            # softmax denominators for the 128 rows of i-tile `it`: ones-
            # moving matmul accumulating over attnT windows -> [128,1] PSUM.
            rp = psum.tile([P, 1], F32, tag="mm", bufs=8, name="rp")
            for jt in range(it + 1):
                nc.tensor.matmul(
                    rp,
                    att_win(jt, it),
                    ones[:, 0:1],
                    start=(jt == 0),
                    stop=(jt == it),
                )
            rpt = rpool.tile([P, 1], F32, tag="rpt", bufs=3, name="rpt")
            nc.vector.reciprocal(out=rpt, in_=rp)

            ysb = ypool.tile([P, S], F32, tag="y", bufs=3, name="ysb")
            # On the last i-tile, normalize + store in 128-col chunks so the
            # final DMA (and its completion wait) covers 32 KB instead of
            # 256 KB — trims the post-matmul tail.
            CH = P if it == NT - 1 else NB
            for eb in range(2):
                py = psum.tile([P, NB], F32, tag="mm", bufs=8, name="py")
                for jt in range(it + 1):
                    nc.tensor.matmul(
                        py,
                        att_win(jt, it),
                        v2sb[:, jt * D + eb * NB : jt * D + (eb + 1) * NB],
                        start=(jt == 0),
                        stop=(jt == it),
                    )
                for c0 in range(eb * NB, (eb + 1) * NB, CH):
                    nc.vector.tensor_scalar_mul(
                        ysb[:, c0 : c0 + CH], py[:, c0 - eb * NB : c0 - eb * NB + CH], rpt
                    )
                    nc.sync.dma_start(
                        y_d[it * P : (it + 1) * P, c0 : c0 + CH],
                        ysb[:, c0 : c0 + CH],
                    )

        # Software pipeline: scores one j-tile ahead of the r/Y consumer so
        # the ACT exp and gpsimd select latencies hide under the next tile's
        # score matmuls.
        scores_step(0)
        for jt in range(1, NT):
            scores_step(jt)
            ry_step(jt - 1)
        ry_step(NT - 1)

    nc.finalize()
    return nc


_CACHED_NC = None


def kernel(x, wq, wk, wv, wo, _trace=False, _trace_cores=None):
    global LAST_RESULTS, _CACHED_NC
    assert x.shape == (B, S, D)
    if _CACHED_NC is None:
        _CACHED_NC = _build()
    nc = _CACHED_NC

    bf = ml_dtypes.bfloat16
    x = np.asarray(x, dtype=np.float32)
    m_b = np.ascontiguousarray(
        np.asarray(wq, dtype=np.float32).T @ np.asarray(wk, dtype=np.float32)
    ).astype(bf)
    w2_b = np.ascontiguousarray(
        np.asarray(wv, dtype=np.float32).T @ np.asarray(wo, dtype=np.float32).T
    ).astype(bf)
    in_maps = [
        {
            "xT": x[b].T.astype(bf),
            "M": m_b,
            "W2": w2_b,
        }
        for b in range(N_CORES)
    ]

    kw = {}
    if _trace_cores is not None:
        kw["trace_cores"] = _trace_cores
    if _trace:
        res = run_bass_kernel_spmd(
            nc, in_maps, core_ids=list(range(N_CORES)), trace=True, **kw
        )
    else:
        # Force-disable tracing: the trace path needs an axon NTFF hook that
        # this image's antenv lacks, so a stray BASS_TRACE env would crash.
        prev = os.environ.get("BASS_NEVER_TRACE")
        os.environ["BASS_NEVER_TRACE"] = "1"
        try:
            res = run_bass_kernel_spmd(
                nc, in_maps, core_ids=list(range(N_CORES)), trace=False, **kw
            )
        finally:
            if prev is None:
                os.environ.pop("BASS_NEVER_TRACE", None)
            else:
                os.environ["BASS_NEVER_TRACE"] = prev
    LAST_RESULTS = res
    out = np.stack([res.results[b]["y"] for b in range(N_CORES)], axis=0)
    return out.astype(np.float32, copy=False)


# revision 11
# speedup vs baseline: 1.0219x; 1.0219x over previous
"""Causal single-head self-attention (B=8, S=1024, D=1024, f32) on 8 TRN2 cores.

Sharding: data-parallel over batch (1 batch element per core). The algebra is
restructured on the host to remove two of the five device GEMMs:

  scores = x wq^T wk x^T = x M x^T          with M  = wq^T wk   (host GEMM)
  y      = attn x wv^T wo^T = attn (x W2)   with W2 = wv^T wo^T (host GEMM)

Per-core dataflow (everything bf16 in SBUF, f32 PSUM accumulation):
  tT[e,i]  = M^T-slab contraction with xT        (65536 moving rows)
  v2[s,e]  = x @ W2                              (65536 rows)
  per j-tile jt (single causal pass, software-pipelined one step):
    scoresT[j,i] = x t^T  for i >= 128*jt        (36864 rows total)
    attnT = exp(scoresT/32)  (ACT, PSUM->SBUF bf16; affine_select masks the
                              diagonal 128x128 block; no other masking needed
                              because tiles are trimmed exactly to the causal
                              boundary)
    r[i]  = ones-moving matmul over attnT        ([128,1] PSUM, no transpose
                                                  or DRAM round trip)
    y[i,e] = sum_jt attnT^T @ v2, * 1/r fused into the PSUM->SBUF copy,
             streamed to DRAM per 128-row slab   (36864 rows)

bf16 matmuls run 1 cyc/row at any width (so causal trimming is exact at 128
granularity), input DMA is half of f32, and the host pre-fusion removes
~55 us of PE work vs the 5-GEMM formulation. Measured vs the fp32 reference
the scheme sits at ~4e-3 scale-relative max error (CPU bit-model), well under
the 2e-2 gate.
"""

import os
import sys

sys.path.insert(0, "/opt/trn_rl_repo")

from contextlib import ExitStack

import ml_dtypes
import numpy as np

import concourse.bass as bass
from concourse import bacc
import concourse.mybir as mybir
import concourse.tile as tile
from concourse.tile import add_dep_helper
from concourse.bass_utils import run_bass_kernel_spmd

B, S, D = 8, 1024, 1024
P = 128          # partition / stationary tile size
NB = 512         # moving-operand block (= 1 PSUM bank of f32)
NT = S // P      # 8 tiles of 128
SCALE = 1.0 / np.sqrt(float(D))

F32 = mybir.dt.float32
BF16 = mybir.dt.bfloat16

N_CORES = 8

# attnT[jt] has width 1024 - 128*jt (columns i >= 128*jt); packed offsets.
AW = [S - P * jt for jt in range(NT)]
AOFF = [sum(AW[:jt]) for jt in range(NT)]
ATOT = sum(AW)  # 4608

LAST_RESULTS = None  # BassKernelResults of the most recent run (for test.py)


def _build():
    nc = bacc.Bacc("TRN2", target_bir_lowering=False, debug=False)

    xT_d = nc.dram_tensor("xT", [D, S], BF16, kind="ExternalInput").ap()
    m_d = nc.dram_tensor("M", [D, D], BF16, kind="ExternalInput").ap()
    w2_d = nc.dram_tensor("W2", [D, D], BF16, kind="ExternalInput").ap()
    y_d = nc.dram_tensor("y", [S, D], F32, kind="ExternalOutput").ap()

    # SBUF layout of a 1024x1024 matrix: big tile [128, 8192] where column
    # range t*1024..(t+1)*1024 holds DRAM rows t*128..(t+1)*128. Each
    # dma_start costs ~600 ns of sync-queue issue time, so load multi-slab
    # chunks with a single 3D access pattern instead of per-slab transfers.
    def chunk_load(sbuf_tile, dram_ap, t0, t1, eng=None):
        eng = eng or nc.sync
        return eng.dma_start(
            sbuf_tile[:, t0 * S : t1 * S],
            dram_ap[t0 * P : t1 * P, :].rearrange("(t p) s -> p t s", p=P),
        )

    with tile.TileContext(nc) as tc, ExitStack() as ctx:
        consts = ctx.enter_context(tc.tile_pool(name="consts", bufs=1))
        ones = consts.tile([P, 8], BF16)
        nc.gpsimd.memset(ones, 1.0)
        zbias = consts.tile([P, 1], F32)
        nc.gpsimd.memset(zbias, 0.0)
        junk = consts.tile([P, 256], BF16)
        nc.gpsimd.memset(junk, 0.5)

        psum = ctx.enter_context(tc.tile_pool(name="psum", bufs=8, space="PSUM"))

        xpool = ctx.enter_context(tc.tile_pool(name="xpool", bufs=1))
        mpool = ctx.enter_context(tc.tile_pool(name="mpool", bufs=1))
        tpool = ctx.enter_context(tc.tile_pool(name="tpool", bufs=1))
        w2pool = ctx.enter_context(tc.tile_pool(name="w2pool", bufs=1))
        vpool = ctx.enter_context(tc.tile_pool(name="vpool", bufs=1))
        apool = ctx.enter_context(tc.tile_pool(name="apool", bufs=1))
        ypool = ctx.enter_context(tc.tile_pool(name="ypool", bufs=3))
        rpool = ctx.enter_context(tc.tile_pool(name="rpool", bufs=3))

        xsb = xpool.tile([P, NT * S], BF16, name="xsb")
        msb = mpool.tile([P, NT * D], BF16, name="msb")
        tsb = tpool.tile([P, NT * S], BF16, name="tsb")
        w2sb = w2pool.tile([P, NT * D], BF16, name="w2sb")
        v2sb = vpool.tile([P, NT * D], BF16, name="v2sb")
        atile = apool.tile([P, ATOT], BF16, name="atile")

        # HAM warmup: keep the PE array busy while the first chunks are in
        # flight so the clock gate is ramped when the real waves start.
        for _ in range(6):
            pw = psum.tile([8, 256], F32, tag="mm", bufs=8, name="pw")
            nc.tensor.matmul(pw, ones, junk, start=True, stop=True)

        # x chunks issue on the sync queue while M chunks issue in parallel
        # on the Activation HWDGE queue (it is idle until the attention
        # phase), halving the ~600ns-per-issue serialization of the ramp.
        # Front chunks are single slabs so the first matmuls start early.
        for t0, t1 in ((0, 1), (1, 2), (2, 4), (4, 6), (6, 8)):
            chunk_load(xsb, xT_d, t0, t1, eng=nc.sync)
            chunk_load(msb, m_d, t0, t1, eng=nc.scalar)

        def mm_t(pt, et, sb, dt):
            nc.tensor.matmul(
                pt,
                msb[:, dt * D + et * P : dt * D + (et + 1) * P],
                xsb[:, dt * S + sb * NB : dt * S + (sb + 1) * NB],
                start=(dt == 0),
                stop=(dt == NT - 1),
            )

        t_copies = {}

        def t_copy(pt, et, sb):
            inst = nc.vector.tensor_copy(
                out=tsb[:, et * S + sb * NB : et * S + (sb + 1) * NB],
                in_=pt,
            )
            t_copies[(et, sb)] = inst
            return inst

        # Phase 0 of tT: 8 PSUM groups accumulated d-tile-major.
        groups = [(et, sb) for et in range(4) for sb in range(2)]
        pts = {}
        for g in groups:
            pts[g] = psum.tile([P, NB], F32, tag="mm", bufs=8, name="pt")
        for dt in range(NT):
            for (et, sb) in groups:
                mm_t(pts[(et, sb)], et, sb, dt)
        for (et, sb) in groups:
            t_copy(pts[(et, sb)], et, sb)

        # Remaining e-tiles of tT, standard order.
        for et in range(4, NT):
            for sb in range(2):
                pt = psum.tile([P, NB], F32, tag="mm", bufs=8, name="pt")
                for dt in range(NT):
                    mm_t(pt, et, sb, dt)
                t_copy(pt, et, sb)

        # W2 prefetch in two chunks spread across the tT phase.
        for i, (t0, t1) in enumerate(((0, 4), (4, 8))):
            dma = chunk_load(w2sb, w2_d, t0, t1, eng=nc.scalar)
            anchor = t_copies.get((1 + 2 * i, 0))
            if anchor is not None:
                add_dep_helper(dma.ins, anchor.ins, reason="w2 prefetch pacing")

        # v2[s, e] = x @ W2: stationary xT s-tile, moving W2 slab.
        for st in range(NT):
            for eb in range(2):
                pt = psum.tile([P, NB], F32, tag="mm", bufs=8, name="pt")
                for dt in range(NT):
                    nc.tensor.matmul(
                        pt,
                        xsb[:, dt * S + st * P : dt * S + (st + 1) * P],
                        w2sb[:, dt * D + eb * NB : dt * D + (eb + 1) * NB],
                        start=(dt == 0),
                        stop=(dt == NT - 1),
                    )
                nc.vector.tensor_copy(
                    out=v2sb[:, st * D + eb * NB : st * D + (eb + 1) * NB],
                    in_=pt,
                )

        def att_win(jt, it):
            return atile[:, AOFF[jt] + (it - jt) * P : AOFF[jt] + (it - jt + 1) * P]

        def scores_step(jt):
            # scoresT[j in jt, i >= 128*jt], trimmed exactly to the causal
            # boundary; exp into the packed attnT tile; mask the diagonal
            # 128x128 block.
            i0 = jt * P
            c0 = i0
            while c0 < S:
                cw = min(NB, S - c0)
                ps = psum.tile([P, cw], F32, tag="mm", bufs=8, name="ps")
                for et in range(NT):
                    nc.tensor.matmul(
                        ps,
                        xsb[:, et * S + i0 : et * S + i0 + P],
                        tsb[:, et * S + c0 : et * S + c0 + cw],
                        start=(et == 0),
                        stop=(et == NT - 1),
                    )
                nc.scalar.activation(
                    out=atile[:, AOFF[jt] + (c0 - i0) : AOFF[jt] + (c0 - i0) + cw],
                    in_=ps,
                    func=mybir.ActivationFunctionType.Exp,
                    bias=zbias,
                    scale=SCALE,
                )
                c0 += cw
            # keep where i_local - j_local >= 0 within the diagonal block
            nc.gpsimd.affine_select(
                out=atile[:, AOFF[jt] : AOFF[jt] + P],
                in_=atile[:, AOFF[jt] : AOFF[jt] + P],
                compare_op=mybir.AluOpType.is_ge,
                fill=0.0,
                base=0,
                pattern=[[1, P]],
                channel_multiplier=-1,
            )

        def ry_step(it):
            # softmax denominators for the 128 rows of i-tile `it`: ones-
            # moving matmul accumulating over attnT windows -> [128,1] PSUM.
            rp = psum.tile([P, 1], F32, tag="mm", bufs=8, name="rp")
            for jt in range(it + 1):
                nc.tensor.matmul(
                    rp,
                    att_win(jt, it),
                    ones[:, 0:1],
                    start=(jt == 0),
                    stop=(jt == it),
                )
            rpt = rpool.tile([P, 1], F32, tag="rpt", bufs=3, name="rpt")
            nc.vector.reciprocal(out=rpt, in_=rp)

            ysb = ypool.tile([P, S], F32, tag="y", bufs=3, name="ysb")
            for eb in range(2):
                py = psum.tile([P, NB], F32, tag="mm", bufs=8, name="py")
                for jt in range(it + 1):
                    nc.tensor.matmul(
                        py,
                        att_win(jt, it),
                        v2sb[:, jt * D + eb * NB : jt * D + (eb + 1) * NB],
                        start=(jt == 0),
                        stop=(jt == it),
                    )
                nc.vector.tensor_scalar_mul(
                    ysb[:, eb * NB : (eb + 1) * NB], py, rpt
                )
                # alternate y-store issues between the two HWDGE queues so
                # they don't serialize behind each other's ~600ns issue cost
                (nc.scalar if eb == 0 else nc.sync).dma_start(
                    y_d[it * P : (it + 1) * P, eb * NB : (eb + 1) * NB],
                    ysb[:, eb * NB : (eb + 1) * NB],
                )

        # Software pipeline: scores one j-tile ahead of the r/Y consumer so
        # the ACT exp and gpsimd select latencies hide under the next tile's
        # score matmuls.
        scores_step(0)
        for jt in range(1, NT):
            scores_step(jt)
            ry_step(jt - 1)
        ry_step(NT - 1)

    nc.finalize()
    return nc


_CACHED_NC = None


def kernel(x, wq, wk, wv, wo, _trace=False, _trace_cores=None):
    global LAST_RESULTS, _CACHED_NC
    assert x.shape == (B, S, D)
    if _CACHED_NC is None:
        _CACHED_NC = _build()
    nc = _CACHED_NC

    bf = ml_dtypes.bfloat16
    x = np.asarray(x, dtype=np.float32)
    m_b = np.ascontiguousarray(
        np.asarray(wq, dtype=np.float32).T @ np.asarray(wk, dtype=np.float32)
    ).astype(bf)
    w2_b = np.ascontiguousarray(
        np.asarray(wv, dtype=np.float32).T @ np.asarray(wo, dtype=np.float32).T
    ).astype(bf)
    in_maps = [
        {
            "xT": x[b].T.astype(bf),
            "M": m_b,
            "W2": w2_b,
        }
        for b in range(N_CORES)
    ]

    kw = {}
    if _trace_cores is not None:
        kw["trace_cores"] = _trace_cores
    if _trace:
        res = run_bass_kernel_spmd(
            nc, in_maps, core_ids=list(range(N_CORES)), trace=True, **kw
        )
    else:
        # Force-disable tracing: the trace path needs an axon NTFF hook that
        # this image's antenv lacks, so a stray BASS_TRACE env would crash.
        prev = os.environ.get("BASS_NEVER_TRACE")
        os.environ["BASS_NEVER_TRACE"] = "1"
        try:
            res = run_bass_kernel_spmd(
                nc, in_maps, core_ids=list(range(N_CORES)), trace=False, **kw
            )
        finally:
            if prev is None:
                os.environ.pop("BASS_NEVER_TRACE", None)
            else:
                os.environ["BASS_NEVER_TRACE"] = prev
    LAST_RESULTS = res
    out = np.stack([res.results[b]["y"] for b in range(N_CORES)], axis=0)
    return out.astype(np.float32, copy=False)
